# revision 10
# baseline (speedup 1.0000x reference)
"""Trainium2 Bass kernel for multi-head attention (B=4, S=2048, D=1024, H=16).

Sharding: 8 cores = 4 batches x 2 head-groups. Core i handles batch i//2,
heads (i%2)*8 .. (i%2)*8+8. Q/K/V projections are column-parallel, the output
projection is row-parallel; the two partial outputs per batch are summed on
the host (plus bo). Biases bq/bk/bv are zero per the problem spec and are
not applied on device.

Per-core kernel (all matmuls bf16, f32 accumulation):
  - inputs arrive pre-transposed as x^T [1024, 2048] so activations sit
    channels-on-partitions (what the PE's lhsT/rhs layouts want).
  - QH^T/KH^T [dims, tokens] via out = Wq^T-slice.T @ x^T
  - VH [tokens, dims] via out = x^T.T @ Wv-slice, stored per key-tile with a
    ones column appended per head ([128, 65] tiles) so the PV matmul also
    produces softmax row-sums in PSUM row 64.
  - scores computed transposed S^T[k, q] = KH^T.T @ QH^T with two heads
    row-packed in the PE array (each head only needs K=64 contraction rows).
  - exp on ScalarE straight out of PSUM with the 1/sqrt(d_model) scale fused.
  - PV accumulates over 16 key tiles into a [65, 512] PSUM bank per head.
  - normalization: reciprocal_approx_fast of row 64, broadcast across 64
    partitions with a K=1 matmul, one DVE multiply -> O^T bf16.
  - output projection computed transposed: out^T = Wo-slice.T.T @ O^T,
    DMA'd out as [1024, 2048] f32; host transposes + sums the pair.
"""

import sys

for _p in ("/opt/trn_rl_repo", "/root/.axon_site"):
    if _p not in sys.path:
        sys.path.insert(0, _p)

# Benign shim: if antenv.axon_hooks is missing (as in this image), provide a
# null hook module so run_bass_kernel_spmd(trace=True) degrades instead of
# crashing. Harmless when tracing is off.
def _ensure_axon_hooks():
    import types
    if "antenv.axon_hooks" in sys.modules:
        return
    try:
        import antenv  # noqa
        from antenv import axon_hooks  # noqa
        return  # real one exists
    except ImportError:
        pass
    mod = types.ModuleType("antenv.axon_hooks")
    _h = {"hook": None}
    mod.set_axon_ntff_profile_hook = lambda h: _h.__setitem__("hook", h)
    mod.get_axon_ntff_profile_hook = lambda: _h["hook"]
    sys.modules["antenv.axon_hooks"] = mod
    try:
        import antenv
        antenv.axon_hooks = mod
    except ImportError:
        pass

_ensure_axon_hooks()

import numpy as np
import ml_dtypes

import concourse.bacc as bacc
import concourse.mybir as mybir
import concourse.tile as tile
from concourse.bass_utils import run_bass_kernel_spmd

BF16 = mybir.dt.bfloat16
F32 = mybir.dt.float32

S = 2048          # sequence length
C = 1024          # d_model
DL = 512          # local head dims (8 heads x 64)
NPAIR = 4         # head pairs per core (2 heads each)
HEAD_D = 64
NKT = S // 128    # 16 key tiles
NQB = S // 512    # 4 query blocks
NCT = C // 128    # 8 channel tiles
SCALE = 1.0 / 32.0  # 1/sqrt(d_model)

_CACHE = {}


def build_nc():
    nc = bacc.Bacc("TRN2")

    xq_ext = nc.declare_dram_parameter("xq_t", [C, S], BF16, isOutput=False)
    xk_ext = nc.declare_dram_parameter("xk_t", [C, S], BF16, isOutput=False)
    xv_ext = nc.declare_dram_parameter("xv_t", [C, S], BF16, isOutput=False)
    wq_ext = nc.declare_dram_parameter("wq", [C, DL], BF16, isOutput=False)
    wk_ext = nc.declare_dram_parameter("wk", [C, DL], BF16, isOutput=False)
    wv_ext = nc.declare_dram_parameter("wv", [C, DL], BF16, isOutput=False)
    wo_ext = nc.declare_dram_parameter("wo", [DL, C], BF16, isOutput=False)
    out_ext = nc.declare_dram_parameter("out", [C, S], F32, isOutput=True)

    with tile.TileContext(nc) as tc:
        with (
            tc.tile_pool(name="wpool", bufs=1) as wpool,
            tc.tile_pool(name="xt", bufs=16) as xtp,
            tc.tile_pool(name="qk", bufs=1) as qkp,
            tc.tile_pool(name="vh", bufs=1) as vhp,
            tc.tile_pool(name="e", bufs=13) as ep,
            tc.tile_pool(name="ou", bufs=2) as oup,
            tc.tile_pool(name="ot", bufs=1) as otp,
            tc.tile_pool(name="rr", bufs=1) as rrp,
            tc.tile_pool(name="obt", bufs=2) as obtp,
            tc.tile_pool(name="outs", bufs=1) as outsp,
            tc.tile_pool(name="const", bufs=1) as constp,
            tc.tile_pool(name="pss", bufs=2, space="PSUM") as pss,
            tc.tile_pool(name="pso", bufs=2, space="PSUM") as pso,
            tc.tile_pool(name="pp", bufs=1, space="PSUM") as pp,
            tc.tile_pool(name="pv", bufs=1, space="PSUM") as pvp,
        ):
            ones64 = constp.tile([1, 64], F32, tag="ones")
            nc.gpsimd.memset(ones64[:], 1.0)

            # ---- weight + input DMAs -------------------------------------
            wq_sb = []
            wk_sb = []
            wv_sb = []
            for ct in range(NCT):
                t = wpool.tile([128, DL], BF16, tag=f"wq{ct}", name=f"wq{ct}")
                nc.sync.dma_start(t[:], wq_ext[128 * ct:128 * (ct + 1), :])
                wq_sb.append(t)
            for ct in range(NCT):
                t = wpool.tile([128, DL], BF16, tag=f"wk{ct}", name=f"wk{ct}")
                nc.sync.dma_start(t[:], wk_ext[128 * ct:128 * (ct + 1), :])
                wk_sb.append(t)
            xq_sb = []
            for ct in range(NCT):
                t = xtp.tile([128, S], BF16, tag="xt", name="xt")
                nc.sync.dma_start(t[:], xq_ext[128 * ct:128 * (ct + 1), :])
                xq_sb.append(t)
            xk_sb = []
            for ct in range(NCT):
                t = xtp.tile([128, S], BF16, tag="xt", name="xt")
                nc.sync.dma_start(t[:], xk_ext[128 * ct:128 * (ct + 1), :])
                xk_sb.append(t)
            xv_sb = []
            for ct in range(NCT):
                t = xtp.tile([128, S], BF16, tag="xt", name="xt")
                nc.sync.dma_start(t[:], xv_ext[128 * ct:128 * (ct + 1), :])
                xv_sb.append(t)
            for ct in range(NCT):
                t = wpool.tile([128, DL], BF16, tag=f"wv{ct}", name=f"wv{ct}")
                nc.sync.dma_start(t[:], wv_ext[128 * ct:128 * (ct + 1), :])
                wv_sb.append(t)
            wo_sb = []
            for dt in range(4):
                t = wpool.tile([128, C], BF16, tag=f"wo{dt}", name=f"wo{dt}")
                nc.sync.dma_start(t[:], wo_ext[128 * dt:128 * (dt + 1), :])
                wo_sb.append(t)

            qht = [qkp.tile([128, S], BF16, tag=f"qht{p}", name=f"qht{p}") for p in range(NPAIR)]
            kht = [qkp.tile([128, S], BF16, tag=f"kht{p}", name=f"kht{p}") for p in range(NPAIR)]
            # vh[p][kt]: [128 keys, 130] = [A V(64) | A ones | B V(64) | B ones]
            vh = [[vhp.tile([128, 130], BF16, tag=f"vh{p}_{kt}", name=f"vh{p}_{kt}")
                   for kt in range(NKT)] for p in range(NPAIR)]
            ot = [otp.tile([128, S], BF16, tag=f"ot{p}", name=f"ot{p}") for p in range(NPAIR)]

            def proj_qk(p):
                # QH^T/KH^T rows for pair p: out[d, t] = sum_c W[c, d] x^T[c, t]
                for (w_sb, x_sb, dst) in ((wq_sb, xq_sb, qht[p]), (wk_sb, xk_sb, kht[p])):
                    for tb in range(4):
                        ps = pp.tile([128, 512], F32, tag="pp", name="pp")
                        for ct in range(NCT):
                            nc.tensor.matmul(
                                ps[:],
                                w_sb[ct][:, 128 * p:128 * (p + 1)],
                                x_sb[ct][:, 512 * tb:512 * (tb + 1)],
                                start=(ct == 0), stop=(ct == NCT - 1),
                            )
                        nc.vector.tensor_copy(dst[:, 512 * tb:512 * (tb + 1)], ps[:])

            def proj_v():
                # VH[t, d] for all pairs: out[t, d] = sum_c x^T[c, t] Wv[c, d]
                for tt in range(NKT):
                    ps = pvp.tile([128, 512], F32, tag="pv", name="pv")
                    for ct in range(NCT):
                        nc.tensor.matmul(
                            ps[:],
                            xv_sb[ct][:, 128 * tt:128 * (tt + 1)],
                            wv_sb[ct][:],
                            start=(ct == 0), stop=(ct == NCT - 1),
                        )
                    for p in range(NPAIR):
                        dst = vh[p][tt]
                        nc.vector.tensor_copy(dst[:, 0:64], ps[:, 128 * p:128 * p + 64])
                        nc.vector.tensor_copy(dst[:, 65:129], ps[:, 128 * p + 64:128 * p + 128])
                        nc.gpsimd.memset(dst[:, 64:65], 1.0)
                        nc.gpsimd.memset(dst[:, 129:130], 1.0)

            def attention(p):
                for qb in range(NQB):
                    po_a = pso.tile([65, 512], F32, tag="pso", name="pso")
                    po_b = pso.tile([65, 512], F32, tag="pso", name="pso")
                    ou_a = oup.tile([65, 512], F32, tag="ou", name="ou")
                    ou_b = oup.tile([65, 512], F32, tag="ou", name="ou")
                    for kt in range(NKT):
                        ps = pss.tile([128, 1024], F32, tag="pss", name="pss")
                        # scores transposed: out[k, q]; two heads row-packed
                        nc.tensor.matmul(
                            ps[:, 0:512],
                            kht[p][0:64, 128 * kt:128 * (kt + 1)],
                            qht[p][0:64, 512 * qb:512 * (qb + 1)],
                            start=True, stop=True,
                        )
                        nc.tensor.matmul(
                            ps[:, 512:1024],
                            kht[p][64:128, 128 * kt:128 * (kt + 1)],
                            qht[p][64:128, 512 * qb:512 * (qb + 1)],
                            start=True, stop=True,
                        )
                        e = ep.tile([128, 1024], BF16, tag="e", name="e")
                        nc.scalar.activation(
                            e[:], ps[:], mybir.ActivationFunctionType.Exp, scale=SCALE
                        )
                        nc.tensor.matmul(
                            po_a[:], vh[p][kt][:, 0:65], e[:, 0:512],
                            start=(kt == 0), stop=(kt % 4 == 3),
                            skip_group_check=(kt != 0),
                        )
                        nc.tensor.matmul(
                            po_b[:], vh[p][kt][:, 65:130], e[:, 512:1024],
                            start=(kt == 0), stop=(kt % 4 == 3),
                            skip_group_check=(kt != 0),
                        )
                    nc.vector.tensor_copy(ou_a[:], po_a[:])
                    nc.vector.tensor_copy(ou_b[:], po_b[:])
                    for h, ou in ((0, ou_a), (1, ou_b)):
                        # row 64 holds the softmax denominators; DVE lanes are
                        # partition-hardwired, so shift row 64 -> row 0 via DMA
                        rr = rrp.tile([1, 512], F32, tag="rr", name="rr")
                        nc.sync.dma_start(rr[:], ou[64:65, :])
                        rrv = rrp.tile([1, 512], F32, tag="rrv", name="rrv")
                        nc.vector.reciprocal_approx_fast(rrv[:], rr[:])
                        rb = pvp.tile([64, 512], F32, tag="pv", name="rb")
                        nc.tensor.matmul(rb[:], ones64[:], rrv[:], start=True, stop=True)
                        if h == 0:
                            nc.vector.tensor_mul(
                                ot[p][0:64, 512 * qb:512 * (qb + 1)],
                                rb[:], ou[0:64, :],
                            )
                        else:
                            obt = obtp.tile([64, 512], BF16, tag="obt", name="obt")
                            nc.vector.tensor_mul(obt[:], rb[:], ou[0:64, :])
                            nc.sync.dma_start(
                                ot[p][64:128, 512 * qb:512 * (qb + 1)], obt[:]
                            )

            def oproj():
                # out^T[e, t] = sum_d Wo[d, e] O^T[d, t]
                for et in range(8):
                    stage = outsp.tile([128, S], F32, tag="outs", name="outs")
                    for tb in range(4):
                        ps = pp.tile([128, 512], F32, tag="pp", name="pp")
                        for dt in range(4):
                            nc.tensor.matmul(
                                ps[:],
                                wo_sb[dt][:, 128 * et:128 * (et + 1)],
                                ot[dt][:, 512 * tb:512 * (tb + 1)],
                                start=(dt == 0), stop=(dt == 3),
                            )
                        nc.vector.tensor_copy(stage[:, 512 * tb:512 * (tb + 1)], ps[:])
                    nc.sync.dma_start(out_ext[128 * et:128 * (et + 1), :], stage[:])

            proj_qk(0)
            proj_v()
            for p in range(NPAIR):
                if p > 0:
                    proj_qk(p)
                attention(p)
            oproj()

    nc.compile()
    return nc


def _shard_inputs(q, k, v, Wq, Wk, Wv, Wo):
    bf = ml_dtypes.bfloat16
    in_maps = []
    w_cache = {}
    for g in range(2):
        w_cache[g] = {
            "wq": np.ascontiguousarray(Wq[:, DL * g:DL * (g + 1)]).astype(bf),
            "wk": np.ascontiguousarray(Wk[:, DL * g:DL * (g + 1)]).astype(bf),
            "wv": np.ascontiguousarray(Wv[:, DL * g:DL * (g + 1)]).astype(bf),
            "wo": np.ascontiguousarray(Wo[DL * g:DL * (g + 1), :]).astype(bf),
        }
    xt_cache = {}
    for b in range(4):
        xt_cache[b] = {
            "xq_t": np.ascontiguousarray(q[b].T).astype(bf),
            "xk_t": np.ascontiguousarray(k[b].T).astype(bf),
            "xv_t": np.ascontiguousarray(v[b].T).astype(bf),
        }
    for i in range(8):
        b, g = i // 2, i % 2
        m = {}
        m.update(xt_cache[b])
        m.update(w_cache[g])
        in_maps.append(m)
    return in_maps


def run_sharded(inputs, trace=False, **kw):
    if "nc" not in _CACHE:
        _CACHE["nc"] = build_nc()
    nc = _CACHE["nc"]
    in_maps = _shard_inputs(
        inputs["q"], inputs["k"], inputs["v"],
        inputs["Wq"], inputs["Wk"], inputs["Wv"], inputs["Wo"],
    )
    res = run_bass_kernel_spmd(nc, in_maps, list(range(8)), trace=trace, **kw)
    outs = [np.asarray(res.results[i]["out"], dtype=np.float32) for i in range(8)]
    bo = np.asarray(inputs["bo"], dtype=np.float32)
    out = np.stack([(outs[2 * b] + outs[2 * b + 1]).T + bo for b in range(4)])
    return out, res


def kernel(q, k, v, Wq, bq, Wk, bk, Wv, bv, Wo, bo):
    out, _ = run_sharded({
        "q": np.asarray(q), "k": np.asarray(k), "v": np.asarray(v),
        "Wq": np.asarray(Wq), "Wk": np.asarray(Wk), "Wv": np.asarray(Wv),
        "Wo": np.asarray(Wo), "bo": np.asarray(bo),
    })
    return out.astype(np.float32)
